# revision 37
# baseline (speedup 1.0000x reference)
"""AdvancedWaveletDecomp Trainium2 kernel.

Host side: per-sample wavelet filter MLPs (tiny), composed depthwise band
matrices, scalar losses. Device side (8 NeuronCores, batch-parallel, 4
samples/core): depthwise wavelet convs as PE band-matmuls, CrossScaleFusion
(gate convs + attention) with fp16 intermediates, fp32 PSUM accumulation.

Self-contained: only needs numpy + the concourse stack available at runtime.
"""

import math
import os

import numpy as np

# ---------------------------------------------------------------- constants
B, C, L = 32, 128, 4096
LEVEL = 3
FL = 9
DIM = 512
REG = 0.01
PAD = (FL - 1) // 2  # 4

NCORES = 8
NS = B // NCORES  # samples per core = 4

NDW = 40           # 104-wide depthwise output tiles per sample (last = 40)
DWT = 104
CSF = 512          # CrossScaleFusion tile width
NCT = L // CSF     # 8 csf tiles per sample
WIN = 128          # transposed window rows (t_in); window j starts at 104j-12
XLEAD = 12         # zero columns before x data in the padded x buffer
XBUF = XLEAD + L + (DWT * (NDW - 1) + WIN - L - XLEAD)  # 12+4096+76 = 4184

_LP = 512          # probe length for band construction
_NE = 48           # edge probes per side


# ---------------------------------------------------------------- host math
def _erf(x):
    try:
        from scipy.special import erf as _serf  # type: ignore
        return _serf(x)
    except Exception:
        return np.vectorize(math.erf)(x)


def _gelu(x):
    return 0.5 * x * (1.0 + _erf(x / math.sqrt(2.0)))


def _corr_rep(v, f):
    """out[..., t] = sum_k f[..., k] * vpad[..., t+k], edge-replicate pad 4.

    v: [..., n], f broadcastable [..., FL] -> out [..., n]
    """
    vp = np.concatenate(
        [np.repeat(v[..., :1], PAD, axis=-1), v, np.repeat(v[..., -1:], PAD, axis=-1)],
        axis=-1,
    )
    n = v.shape[-1]
    out = np.zeros(np.broadcast_shapes(v.shape, f.shape[:-1] + (n,)), dtype=v.dtype)
    for k in range(FL):
        out = out + f[..., k : k + 1] * vp[..., k : k + n]
    return out


def compute_filters(x, stat_w, stat_b, wg1_w, wg1_b, wg2_w, wg2_b):
    """Compute (lo, hi) filters for all 3 levels without materializing approx.

    Uses the identity: column-sum of a replicate-padded correlation is a
    linear function of the input's column sum and its first/last 4 samples.
    Returns lists lo[3], hi[3] each [B, FL] float64.
    """
    xd = x.astype(np.float64)
    S = xd.sum(axis=2)                       # [B, C]
    E = 24
    F = xd[:, :, :E].copy()                  # first E cols
    G = xd[:, :, -E:].copy()                 # last E cols

    sw = stat_w.astype(np.float64)
    w1 = wg1_w.astype(np.float64)
    w2 = wg2_w.astype(np.float64)
    sb = stat_b.astype(np.float64)
    b1 = wg1_b.astype(np.float64)
    b2 = wg2_b.astype(np.float64)

    los, his = [], []
    for lvl in range(LEVEL):
        feat = S / L                         # [B, C]
        h = _gelu(feat @ sw.T + sb)
        h = _gelu(h @ w1.T + b1)
        filt = h @ w2.T + b2                 # [B, 2*FL]
        lo, hi = filt[:, :FL], filt[:, FL:]
        los.append(lo)
        his.append(hi)
        if lvl == LEVEL - 1:
            break
        # window sums W(k): sum over t of pad(approx)[t+k]
        Wk = np.empty(S.shape + (FL,), dtype=np.float64)
        for k in range(FL):
            if k < PAD:
                m = PAD - k
                Wk[..., k] = m * F[..., 0] + S - G[..., E - m :].sum(axis=-1)
            elif k == PAD:
                Wk[..., k] = S
            else:
                m = k - PAD
                Wk[..., k] = S - F[..., :m].sum(axis=-1) + m * G[..., -1]
        lo3 = lo[:, None, :]                 # [B, 1, FL]
        S = (lo3 * Wk).sum(axis=-1)
        # next level edges (left: replicate only on the left side)
        Fp = np.concatenate([np.repeat(F[..., :1], PAD, axis=-1), F], axis=-1)
        n = F.shape[-1] - PAD
        Fn = np.zeros(F.shape[:-1] + (n,), dtype=np.float64)
        for k in range(FL):
            Fn = Fn + lo[:, k][:, None, None] * Fp[..., k : k + n]
        Gp = np.concatenate([G, np.repeat(G[..., -1:], PAD, axis=-1)], axis=-1)
        Gn = np.zeros(F.shape[:-1] + (n,), dtype=np.float64)
        for k in range(FL):
            Gn = Gn + lo[:, k][:, None, None] * Gp[..., k : k + n]
        F, G = Fn, Gn
    return los, his


def _dw_pipeline(v, los, his):
    """Run the 3-level depthwise pipeline on v [B, P, n] with per-sample
    filters. Returns (det1, det2, det3, app3), each [B, P, n]."""
    f = lambda a: a[:, None, :]
    a1 = _corr_rep(v, f(los[0]))
    d1 = _corr_rep(v, f(his[0]))
    a2 = _corr_rep(a1, f(los[1]))
    d2 = _corr_rep(a1, f(his[1]))
    a3 = _corr_rep(a2, f(los[2]))
    d3 = _corr_rep(a2, f(his[2]))
    return d1, d2, d3, a3


def build_bands(los, his):
    """Build windowed band templates per sample (uniform w0 = 104j - 12).

    The device zero-pads x with 12 leading / 76 trailing columns, so window j
    reads xpad[:, 104j : 104j+128] = x[104j-12 : 104j+116] with zeros outside.
        out_o[c, 104j+i] = sum_r xT[104j-12+r, c] * band[r, o*104+i]
    Returns bands [B, 3, 128, 4*104] float64 (kind-major columns; reordered
    to i-major in prepare()). Template 0 -> j=0 (left pad absorbed),
    1 -> j=1..38, 2 -> j=39 (right pad absorbed; cols i>=40 zero).
    """
    npr = WIN * 2 + 1
    probes = np.zeros((1, npr, _LP), dtype=np.float64)
    for q in range(WIN):
        probes[0, q, q] = 1.0
        probes[0, WIN + q, _LP - WIN + q] = 1.0
    probes[0, 2 * WIN, _LP // 2] = 1.0
    probes = np.repeat(probes, B, axis=0)           # [B, npr, LP]
    outs = _dw_pipeline(probes, los, his)           # 4x [B, npr, LP]

    NK = 4
    bands = np.zeros((B, 3, WIN, NK * DWT), dtype=np.float64)
    ii = np.arange(DWT)
    for o in range(NK):
        out = outs[o]
        # impulse at p: out[t] = coef[p - t + 12]  =>  coef[d] = out[p + 12 - d]
        taps = out[:, 2 * WIN, _LP // 2 - 12 : _LP // 2 + 13][:, ::-1]  # [B, 25]
        # interior template: r = s - w0 = (t - 12 + d) - (104j - 12) = i + d
        for d in range(25):
            bands[:, 1, ii + d, o * DWT + ii] = taps[:, d][:, None]
        # j=0: w0 = -12; rows r=s+12, coefficients from left probes e_s
        # (support s <= i+12 <= 115); rows r < 12 hit the zero pad.
        bands[:, 0, 12:, o * DWT : o * DWT + DWT] = out[:, :116, :DWT]
        # j=39: out t = 4056+i (i<40); band[r, i] = M[t, 4044+r]
        #   = out(e~_{76+r})[472+i] for r < 52; rows >= 52 hit the zero pad.
        bands[:, 2, :52, o * DWT : o * DWT + 40] = \
            out[:, WIN + 76 : WIN + 128, _LP - 40 :]
    return bands


def compute_losses(lo_last):
    """ortho loss from the last level's lo filter, matching reference."""
    lo = lo_last.astype(np.float64)
    lo_pad = np.pad(lo, ((0, 0), (1, 1)))
    lo_smooth = np.abs(lo_pad[:, 1:] - lo_pad[:, :-1]).mean()
    lo_n = lo / (np.linalg.norm(lo, axis=1, keepdims=True) + 1e-8)
    shift = 0.0
    for s in (1, 2, 3):
        sh = np.roll(lo_n, s, axis=1)
        shift = shift + np.abs(lo_n[:, :, None] * sh[:, None, :]).mean()
    amp = np.abs((lo_n * lo_n).sum(axis=1) - 1.0).mean()
    return REG * (shift + amp) + 0.1 * lo_smooth


# ------------------------------------------------------- numpy full forward
def forward_numpy(x, stat_w, stat_b, wg1_w, wg1_b, wg2_w, wg2_b,
                  gates_w, gates_b, attn1_w, attn1_b, attn2_w, attn2_b,
                  dtype=np.float64):
    """Pure numpy replication of the reference (for validation)."""
    los, his = compute_filters(x, stat_w, stat_b, wg1_w, wg1_b, wg2_w, wg2_b)
    xd = x.astype(dtype)
    f = lambda a: a[:, None, :].astype(dtype)
    a1 = _corr_rep(xd, f(los[0])); d1 = _corr_rep(xd, f(his[0]))
    a2 = _corr_rep(a1, f(los[1])); d2 = _corr_rep(a1, f(his[1]))
    a3 = _corr_rep(a2, f(los[2])); d3 = _corr_rep(a2, f(his[2]))
    details = [d1, d2, d3]

    def conv1d(xx, w, b, pad):
        w = w.astype(dtype)
        cout, cin, K = w.shape
        xp = np.pad(xx, ((0, 0), (0, 0), (pad, pad)))
        n = xx.shape[2]
        out = np.zeros((xx.shape[0], cout, n), dtype=dtype)
        for k in range(K):
            out += np.einsum("oc,bct->bot", w[:, :, k], xp[:, :, k : k + n])
        return out + b.astype(dtype)[None, :, None]

    sig = lambda z: 1.0 / (1.0 + np.exp(-z))
    current = a3
    enhanced = []
    for i in range(LEVEL - 1, -1, -1):
        det = details[i]
        if i < LEVEL - 1:
            a = conv1d(current, attn1_w[i], attn1_b[i], 0)
            a = 0.5 * a * (1.0 + _erf(a / math.sqrt(2.0)))
            attn = sig(conv1d(a, attn2_w[i], attn2_b[i], 0))
            det = det * attn + det
        gate = sig(conv1d(current, gates_w[i], gates_b[i], 1))
        current = current + gate * det
        enhanced.append(det)
    enhanced.reverse()
    yl = current
    ortho = compute_losses(los[2])
    lo_all = np.stack(los).astype(np.float32).transpose(0, 1, 2)
    hi_all = np.stack(his).astype(np.float32)
    return (yl, enhanced[0], enhanced[1], enhanced[2],
            np.float32(ortho), np.float32(0.0),
            np.stack(los).astype(np.float32), np.stack(his).astype(np.float32))


# ================================================================ device ==
KW = 4100          # per-kind column stride in the staging buffer
DOFF = 2           # data starts at this column (4B alignment for fp16 ops)
SQ2 = 0.7071067811865476

_PROG_CACHE = {}

# engine assignment knobs (tuned via TimelineSim)
CFG = {
    "tt_pool": False,     # t-mults & cur2-add on GPSIMD
    "dp_pool": False,     # dp-STT on GPSIMD
    "yl_pool": False,     # STT is not a valid Pool opcode on HW (walrus)
    "copy_dve_of5": 2,    # dw copies: j%5 < this -> DVE else ACT
    "ring_bufs": 4,
    "dwp": 2, "gp": 2, "a1p": 2, "a2p": 2,
}


def _build_program(k1, k2, k3, a1b_zero):
    """Build the SPMD single-core Bass program. k1/k2/k3 are log2 of the
    det1 / det2 / (det3,app3) scale factors baked into the band inputs."""
    from contextlib import ExitStack

    import concourse.bacc as bacc
    import concourse.bass as bass
    import concourse.tile as tile
    from concourse import mybir

    dt = mybir.dt
    AF = mybir.ActivationFunctionType
    f16, f32 = dt.float16, dt.float32
    s_d1, s_d2, s_3 = 2.0 ** k1, 2.0 ** k2, 2.0 ** k3

    nc = bacc.Bacc("TRN2", target_bir_lowering=False, debug=False)
    x_d = nc.dram_tensor("x16", [NS, C, L], f16, kind="ExternalInput").ap()
    bands_d = nc.dram_tensor("bands", [C, NS * 3 * 416], f16,
                             kind="ExternalInput").ap()
    gw_d = nc.dram_tensor("gw", [C, 9 * C], f16, kind="ExternalInput").ap()
    a1w_d = nc.dram_tensor("a1w", [C, 64], f16, kind="ExternalInput").ap()
    a2w_d = nc.dram_tensor("a2w", [C, 256], f16, kind="ExternalInput").ap()
    gb_d = nc.dram_tensor("gb", [C, 3], f32, kind="ExternalInput").ap()
    a1be_d = nc.dram_tensor("a1be", [C, 2], f32, kind="ExternalInput").ap()
    a1bs_d = nc.dram_tensor("a1bs", [C, 2], f32, kind="ExternalInput").ap()
    a2b_d = nc.dram_tensor("a2b", [C, 2], f32, kind="ExternalInput").ap()
    outs_d = {n: nc.dram_tensor(n, [NS, C, L], f16, kind="ExternalOutput").ap()
              for n in ("yl", "yh1", "yh2", "yh3")}

    with tile.TileContext(nc) as tc, ExitStack() as ctx:
        pool = lambda name, bufs, **kw: ctx.enter_context(
            tc.tile_pool(name=name, bufs=bufs, **kw))
        konst = pool("konst", 1)
        xp = pool("xp", 2)
        xcp = pool("xcp", 1)
        xtp = pool("xtp", 2)
        bigp = pool("bigp", 2)
        curp = pool("curp", 2)
        ring = pool("ring", CFG["ring_bufs"])
        dwp = pool("dwp", CFG["dwp"], space="PSUM")
        gp = pool("gp", CFG["gp"], space="PSUM")
        a1p = pool("a1p", CFG["a1p"], space="PSUM")
        a2p = pool("a2p", CFG["a2p"], space="PSUM")

        def kload(name, shape, dtype, src):
            t = konst.tile(shape, dtype, tag=name)
            nc.sync.dma_start(t[:], src)
            return t

        bands_sb = kload("bands", [C, NS * 3 * 416], f16, bands_d)
        gw_sb = kload("gw", [C, 9 * C], f16, gw_d)
        a1w_sb = kload("a1w", [C, 64], f16, a1w_d)
        a2w_sb = kload("a2w", [C, 256], f16, a2w_d)
        gb_sb = kload("gb", [C, 3], f32, gb_d)
        a1be_sb = kload("a1be", [C, 2], f32, a1be_d)
        a1bs_sb = kload("a1bs", [C, 2], f32, a1bs_d)
        a2b_sb = kload("a2b", [C, 2], f32, a2b_d)

        ALU = mybir.AluOpType
        for s in range(NS):
            x_sb = xp.tile([C, XBUF], f16, tag="x")
            nc.gpsimd.memset(x_sb[:, 0:XLEAD], 0.0)
            nc.gpsimd.memset(x_sb[:, XLEAD + L :], 0.0)
            nc.sync.dma_start(x_sb[:, XLEAD : XLEAD + L], x_d[s])
            big = bigp.tile([C, 4 * KW], f16, tag="big")
            bigv = big[:].rearrange("p (k n) -> p k n", k=4)
            cur2 = curp.tile([C, KW], f16, tag="cur2")
            cur1 = curp.tile([C, KW], f16, tag="cur1")
            for t in (cur2[:], cur1[:]):
                nc.gpsimd.memset(t[:, DOFF - 1 : DOFF], 0.0)
                nc.gpsimd.memset(t[:, DOFF + L : DOFF + L + 1], 0.0)
            nc.gpsimd.memset(bigv[:, :, DOFF - 1 : DOFF], 0.0)
            nc.gpsimd.memset(bigv[:, :, DOFF + L : DOFF + L + 1], 0.0)

            # -------- depthwise pass: overlap-copy + blocked transpose +
            # one band matmul per 104-wide tile (no halo matmuls)
            xcat = xcp.tile([C, NDW * WIN], f16, tag="xcat")
            xcv = xcat[:].rearrange("p (j i) -> p j i", j=NDW)
            # overlapped windows: block j = xbuf[:, 104j : 104j+128]
            xap = x_sb[:]
            ov = bass.AP(xap.tensor, xap.offset,
                         [list(xap.ap[0]), [DWT, NDW], [1, WIN]])
            nc.sync.dma_start(xcv, ov)
            xt = xtp.tile([C, NDW * WIN], f16, tag="xt")
            xtv = xt[:].rearrange("p (j c) -> p j c", j=NDW)
            nc.sync.dma_start_transpose(xtv, xcat[:])
            for j in range(NDW):
                dq = dwp.tile([C, 4 * DWT], f32, tag="dw")
                tmpl = 0 if j == 0 else (2 if j == NDW - 1 else 1)
                nc.tensor.matmul(
                    dq[:], xtv[:, j, :],
                    bands_sb[:, (s * 3 + tmpl) * 416 : (s * 3 + tmpl + 1) * 416],
                    start=True, stop=True)
                wj = DWT if j < NDW - 1 else 40
                src_ap = dq[:, 0 : 4 * wj].rearrange("p (i k) -> p k i", k=4)
                dst = bigv[:, :, DOFF + DWT * j : DOFF + DWT * j + wj]
                if j % 5 < CFG["copy_dve_of5"]:
                    nc.vector.tensor_copy(dst, src_ap)
                else:
                    nc.scalar.copy(dst, src_ap)

            app3 = bigv[:, 3, :]
            dets = [bigv[:, 0, :], bigv[:, 1, :], bigv[:, 2, :]]

            def gate_conv(lvl, curbuf, t0, inv_scale):
                gq = gp.tile([C, CSF], f32, tag="g")
                for k in range(3):
                    nc.tensor.matmul(
                        gq[:], gw_sb[:, (lvl * 3 + k) * C : (lvl * 3 + k + 1) * C],
                        curbuf[:, t0 + k + 1 : t0 + k + 1 + CSF],
                        start=(k == 0), stop=(k == 2))
                gt = ring.tile([C, CSF], f16, tag="gt")
                nc.scalar.activation(gt[:], gq[:], AF.Sigmoid,
                                     bias=gb_sb[:, lvl : lvl + 1],
                                     scale=inv_scale)
                return gt

            # ---------------- level 2 (no attention)
            for i in range(NCT):
                t0 = CSF * i
                g2 = gate_conv(2, app3, t0, 1.0 / s_3)
                t2 = ring.tile([C, CSF], f16, tag="tt")
                vej = nc.gpsimd if CFG["tt_pool"] else nc.vector
                vej.tensor_mul(t2[:], g2[:],
                               dets[2][:, DOFF + t0 : DOFF + t0 + CSF])
                vej.tensor_add(cur2[:, DOFF + t0 : DOFF + t0 + CSF],
                               t2[:], app3[:, DOFF + t0 : DOFF + t0 + CSF])

            # ---------------- levels 1 and 0 (attention + gate)
            # dp overwrites the (dead) det slot in big; yl overwrites app3.
            for lvl, curi, curo, s_in, s_out in (
                    (1, cur2, cur1, s_3, s_d2), (0, cur1, None, s_d2, s_d1)):
                det = dets[lvl]
                asb = None
                for i in range(NCT):
                    t0 = CSF * i
                    if i % 4 == 0:
                        aq = a1p.tile([C, CSF], f32, tag="a1")
                        for g in range(4):
                            tg = CSF * (i + g)
                            nc.tensor.matmul(
                                aq[32 * g : 32 * (g + 1), :],
                                a1w_sb[:, lvl * 32 : (lvl + 1) * 32],
                                curi[:, DOFF + tg : DOFF + tg + CSF],
                                tile_position=(0, 32 * g),
                                start=True, stop=True, skip_group_check=(g > 0))
                        erf_t = ring.tile([C, CSF], f16, tag="erf")
                        nc.scalar.activation(erf_t[:], aq[:], AF.Erf,
                                             bias=a1be_sb[:, lvl : lvl + 1],
                                             scale=SQ2 / s_in)
                        asb = ring.tile([C, CSF], f16, tag="asb")
                        if a1b_zero:
                            nc.vector.scalar_tensor_tensor(
                                asb[:], erf_t[:], 1.0, aq[:],
                                op0=ALU.add, op1=ALU.mult)
                        else:
                            pre2 = ring.tile([C, CSF], f32, tag="pre2")
                            nc.vector.tensor_scalar(
                                pre2[:], aq[:], a1bs_sb[:, lvl : lvl + 1], None,
                                op0=ALU.add)
                            nc.vector.scalar_tensor_tensor(
                                asb[:], erf_t[:], 1.0, pre2[:],
                                op0=ALU.add, op1=ALU.mult)
                    gx = i % 4
                    a2q = a2p.tile([C, CSF], f32, tag="a2")
                    nc.tensor.matmul(a2q[:],
                                     a2w_sb[32 * gx : 32 * (gx + 1),
                                            lvl * C : (lvl + 1) * C],
                                     asb[32 * gx : 32 * (gx + 1), :],
                                     tile_position=(32 * gx, 0),
                                     start=True, stop=True)
                    attnw = ring.tile([C, CSF], f16, tag="attnw")
                    nc.scalar.activation(attnw[:], a2q[:], AF.Sigmoid,
                                         bias=a2b_sb[:, lvl : lvl + 1],
                                         scale=1.0 / s_in)
                    dp = det[:, DOFF + t0 : DOFF + t0 + CSF]
                    (nc.gpsimd if CFG["dp_pool"] else nc.vector).scalar_tensor_tensor(
                        dp, attnw[:], 1.0, dp,
                        op0=ALU.add, op1=ALU.mult)
                    g1 = gate_conv(lvl, curi, t0, 1.0 / s_in)
                    t1 = ring.tile([C, CSF], f16, tag="tt")
                    (nc.gpsimd if CFG["tt_pool"] else nc.vector).tensor_mul(
                        t1[:], g1[:], dp)
                    if lvl == 1:
                        nc.vector.scalar_tensor_tensor(
                            curo[:, DOFF + t0 : DOFF + t0 + CSF],
                            curi[:, DOFF + t0 : DOFF + t0 + CSF], s_out / s_in,
                            t1[:], op0=ALU.mult, op1=ALU.add)
                    else:
                        (nc.gpsimd if CFG["yl_pool"] else nc.vector).scalar_tensor_tensor(
                            app3[:, DOFF + t0 : DOFF + t0 + CSF],
                            curi[:, DOFF + t0 : DOFF + t0 + CSF],
                            s_out / s_in, t1[:], op0=ALU.mult, op1=ALU.add)
            nc.sync.dma_start(outs_d["yl"][s], app3[:, DOFF : DOFF + L])
            nc.sync.dma_start(outs_d["yh1"][s], dets[0][:, DOFF : DOFF + L])
            nc.sync.dma_start(outs_d["yh2"][s], dets[1][:, DOFF : DOFF + L])
            nc.sync.dma_start(outs_d["yh3"][s], dets[2][:, DOFF : DOFF + L])

    nc.compile()
    return nc


def _get_program(k1, k2, k3, a1b_zero):
    key = (k1, k2, k3, a1b_zero)
    if key not in _PROG_CACHE:
        _PROG_CACHE[key] = _build_program(k1, k2, k3, a1b_zero)
    return _PROG_CACHE[key]


def prepare(x, stat_w, stat_b, wg1_w, wg1_b, wg2_w, wg2_b,
            gates_w, gates_b, attn1_w, attn1_b, attn2_w, attn2_b):
    """Host-side prep: filters, losses, scaled bands, weight layouts,
    per-core input maps. Returns (progkey, in_maps, unscale, extras)."""
    x = np.asarray(x)
    los, his = compute_filters(x, stat_w, stat_b, wg1_w, wg1_b, wg2_w, wg2_b)
    ortho = compute_losses(los[2])
    lo_all = np.stack(los).astype(np.float32)
    hi_all = np.stack(his).astype(np.float32)

    bands = build_bands(los, his)                    # [B, 3, 128, 416] f64
    std_x = float(x.std())
    ks = []
    for o in range(4):
        # center column of the interior template holds the full 25-tap set
        tapn = np.sqrt((bands[:, 1, :, o * DWT + 52] ** 2).sum(axis=-1)).max()
        est = max(tapn * std_x * 5.0, 1e-300)
        ks.append(int(np.clip(round(math.log2(2.0 / est)), -100, 100)))
    k1 = ks[0]
    k2 = ks[1]
    k3 = min(ks[2], ks[3])
    scl = np.array([2.0 ** k1, 2.0 ** k2, 2.0 ** k3, 2.0 ** k3])
    bands = bands * np.repeat(scl, DWT)[None, None, None, :]
    # reorder band columns kind-major -> i-major (col = i*4 + kind)
    Bn = bands.shape[0]
    bands = bands.reshape(Bn, 3, WIN, 4, DWT).transpose(0, 1, 2, 4, 3)
    bands = np.ascontiguousarray(bands).reshape(Bn, 3, WIN, 4 * DWT)
    a1b_zero = bool(np.all(np.asarray(attn1_b) == 0.0))

    gw = np.ascontiguousarray(
        np.asarray(gates_w).transpose(2, 0, 3, 1).reshape(C, 9 * C)
    ).astype(np.float16)
    a1w = np.ascontiguousarray(
        np.asarray(attn1_w)[:, :, :, 0].transpose(2, 0, 1).reshape(C, 64)
    ).astype(np.float16)
    a2w_eff = 0.5 * np.asarray(attn2_w)[:, :, :, 0]       # [2, 128, 32]
    a2w = np.ascontiguousarray(
        np.tile(a2w_eff.transpose(2, 0, 1).reshape(32, 2 * C), (4, 1))
    ).astype(np.float16)
    gb = np.ascontiguousarray(np.asarray(gates_b).T).astype(np.float32)
    a2b = np.ascontiguousarray(np.asarray(attn2_b).T).astype(np.float32)
    a1t = np.tile(np.asarray(attn1_b), (1, 4))            # [2, 128]
    a1be = np.ascontiguousarray((a1t * SQ2).T).astype(np.float32)
    a1bs = np.ascontiguousarray(
        (a1t * np.array([[2.0 ** k2], [2.0 ** k3]])).T).astype(np.float32)

    in_maps = []
    for c in range(NCORES):
        sl = slice(c * NS, (c + 1) * NS)
        bsh = bands[sl].transpose(2, 0, 1, 3).reshape(C, NS * 3 * 416)
        in_maps.append({
            "x16": np.ascontiguousarray(x[sl]).astype(np.float16),
            "bands": np.ascontiguousarray(bsh).astype(np.float16),
            "gw": gw, "a1w": a1w, "a2w": a2w, "gb": gb,
            "a1be": a1be, "a1bs": a1bs, "a2b": a2b,
        })
    unscale = {"yl": 2.0 ** -k1, "yh1": 2.0 ** -k1,
               "yh2": 2.0 ** -k2, "yh3": 2.0 ** -k3}
    return (k1, k2, k3, a1b_zero), in_maps, unscale, (ortho, lo_all, hi_all)


def kernel(x, stat_w, stat_b, wg1_w, wg1_b, wg2_w, wg2_b,
           gates_w, gates_b, attn1_w, attn1_b, attn2_w, attn2_b):
    from concourse.bass_utils import run_bass_kernel_spmd

    args = [np.asarray(a) for a in (
        x, stat_w, stat_b, wg1_w, wg1_b, wg2_w, wg2_b,
        gates_w, gates_b, attn1_w, attn1_b, attn2_w, attn2_b)]
    key, in_maps, unscale, (ortho, lo_all, hi_all) = prepare(*args)
    nc = _get_program(*key)
    res = run_bass_kernel_spmd(nc, in_maps, list(range(NCORES)))
    full = {}
    for name in ("yl", "yh1", "yh2", "yh3"):
        parts = [res.results[c][name].astype(np.float32) * unscale[name]
                 for c in range(NCORES)]
        full[name] = np.concatenate(parts, axis=0)
    return (full["yl"], full["yh1"], full["yh2"], full["yh3"],
            np.float32(ortho), np.float32(0.0), lo_all, hi_all)


if __name__ == "__main__":
    print("kernel.py module loaded OK")


# revision 41
# speedup vs baseline: 1.0278x; 1.0278x over previous
"""AdvancedWaveletDecomp Trainium2 kernel.

Host side: per-sample wavelet filter MLPs (tiny), composed depthwise band
matrices, scalar losses. Device side (8 NeuronCores, batch-parallel, 4
samples/core): depthwise wavelet convs as PE band-matmuls, CrossScaleFusion
(gate convs + attention) with fp16 intermediates, fp32 PSUM accumulation.

Self-contained: only needs numpy + the concourse stack available at runtime.
"""

import math
import os

import numpy as np

# ---------------------------------------------------------------- constants
B, C, L = 32, 128, 4096
LEVEL = 3
FL = 9
DIM = 512
REG = 0.01
PAD = (FL - 1) // 2  # 4

NCORES = 8
NS = B // NCORES  # samples per core = 4

NDW = 40           # 104-wide depthwise output tiles per sample (last = 40)
DWT = 104
CSF = 512          # CrossScaleFusion tile width
NCT = L // CSF     # 8 csf tiles per sample
WIN = 128          # transposed window rows (t_in); window j starts at 104j-12
XLEAD = 12         # zero columns before x data in the padded x buffer
XBUF = XLEAD + L + (DWT * (NDW - 1) + WIN - L - XLEAD)  # 12+4096+76 = 4184

_LP = 512          # probe length for band construction
_NE = 48           # edge probes per side


# ---------------------------------------------------------------- host math
def _erf(x):
    try:
        from scipy.special import erf as _serf  # type: ignore
        return _serf(x)
    except Exception:
        return np.vectorize(math.erf)(x)


def _gelu(x):
    return 0.5 * x * (1.0 + _erf(x / math.sqrt(2.0)))


def _corr_rep(v, f):
    """out[..., t] = sum_k f[..., k] * vpad[..., t+k], edge-replicate pad 4.

    v: [..., n], f broadcastable [..., FL] -> out [..., n]
    """
    vp = np.concatenate(
        [np.repeat(v[..., :1], PAD, axis=-1), v, np.repeat(v[..., -1:], PAD, axis=-1)],
        axis=-1,
    )
    n = v.shape[-1]
    out = np.zeros(np.broadcast_shapes(v.shape, f.shape[:-1] + (n,)), dtype=v.dtype)
    for k in range(FL):
        out = out + f[..., k : k + 1] * vp[..., k : k + n]
    return out


def compute_filters(x, stat_w, stat_b, wg1_w, wg1_b, wg2_w, wg2_b):
    """Compute (lo, hi) filters for all 3 levels without materializing approx.

    Uses the identity: column-sum of a replicate-padded correlation is a
    linear function of the input's column sum and its first/last 4 samples.
    Returns lists lo[3], hi[3] each [B, FL] float64.
    """
    xd = x.astype(np.float64)
    S = xd.sum(axis=2)                       # [B, C]
    E = 24
    F = xd[:, :, :E].copy()                  # first E cols
    G = xd[:, :, -E:].copy()                 # last E cols

    sw = stat_w.astype(np.float64)
    w1 = wg1_w.astype(np.float64)
    w2 = wg2_w.astype(np.float64)
    sb = stat_b.astype(np.float64)
    b1 = wg1_b.astype(np.float64)
    b2 = wg2_b.astype(np.float64)

    los, his = [], []
    for lvl in range(LEVEL):
        feat = S / L                         # [B, C]
        h = _gelu(feat @ sw.T + sb)
        h = _gelu(h @ w1.T + b1)
        filt = h @ w2.T + b2                 # [B, 2*FL]
        lo, hi = filt[:, :FL], filt[:, FL:]
        los.append(lo)
        his.append(hi)
        if lvl == LEVEL - 1:
            break
        # window sums W(k): sum over t of pad(approx)[t+k]
        Wk = np.empty(S.shape + (FL,), dtype=np.float64)
        for k in range(FL):
            if k < PAD:
                m = PAD - k
                Wk[..., k] = m * F[..., 0] + S - G[..., E - m :].sum(axis=-1)
            elif k == PAD:
                Wk[..., k] = S
            else:
                m = k - PAD
                Wk[..., k] = S - F[..., :m].sum(axis=-1) + m * G[..., -1]
        lo3 = lo[:, None, :]                 # [B, 1, FL]
        S = (lo3 * Wk).sum(axis=-1)
        # next level edges (left: replicate only on the left side)
        Fp = np.concatenate([np.repeat(F[..., :1], PAD, axis=-1), F], axis=-1)
        n = F.shape[-1] - PAD
        Fn = np.zeros(F.shape[:-1] + (n,), dtype=np.float64)
        for k in range(FL):
            Fn = Fn + lo[:, k][:, None, None] * Fp[..., k : k + n]
        Gp = np.concatenate([G, np.repeat(G[..., -1:], PAD, axis=-1)], axis=-1)
        Gn = np.zeros(F.shape[:-1] + (n,), dtype=np.float64)
        for k in range(FL):
            Gn = Gn + lo[:, k][:, None, None] * Gp[..., k : k + n]
        F, G = Fn, Gn
    return los, his


def _dw_pipeline(v, los, his):
    """Run the 3-level depthwise pipeline on v [B, P, n] with per-sample
    filters. Returns (det1, det2, det3, app3), each [B, P, n]."""
    f = lambda a: a[:, None, :]
    a1 = _corr_rep(v, f(los[0]))
    d1 = _corr_rep(v, f(his[0]))
    a2 = _corr_rep(a1, f(los[1]))
    d2 = _corr_rep(a1, f(his[1]))
    a3 = _corr_rep(a2, f(los[2]))
    d3 = _corr_rep(a2, f(his[2]))
    return d1, d2, d3, a3


def build_bands(los, his):
    """Build windowed band templates per sample (uniform w0 = 104j - 12).

    The device zero-pads x with 12 leading / 76 trailing columns, so window j
    reads xpad[:, 104j : 104j+128] = x[104j-12 : 104j+116] with zeros outside.
        out_o[c, 104j+i] = sum_r xT[104j-12+r, c] * band[r, o*104+i]
    Returns bands [B, 3, 128, 4*104] float64 (kind-major columns; reordered
    to i-major in prepare()). Template 0 -> j=0 (left pad absorbed),
    1 -> j=1..38, 2 -> j=39 (right pad absorbed; cols i>=40 zero).
    """
    npr = WIN * 2 + 1
    probes = np.zeros((1, npr, _LP), dtype=np.float64)
    for q in range(WIN):
        probes[0, q, q] = 1.0
        probes[0, WIN + q, _LP - WIN + q] = 1.0
    probes[0, 2 * WIN, _LP // 2] = 1.0
    probes = np.repeat(probes, B, axis=0)           # [B, npr, LP]
    outs = _dw_pipeline(probes, los, his)           # 4x [B, npr, LP]

    NK = 4
    bands = np.zeros((B, 3, WIN, NK * DWT), dtype=np.float64)
    ii = np.arange(DWT)
    for o in range(NK):
        out = outs[o]
        # impulse at p: out[t] = coef[p - t + 12]  =>  coef[d] = out[p + 12 - d]
        taps = out[:, 2 * WIN, _LP // 2 - 12 : _LP // 2 + 13][:, ::-1]  # [B, 25]
        # interior template: r = s - w0 = (t - 12 + d) - (104j - 12) = i + d
        for d in range(25):
            bands[:, 1, ii + d, o * DWT + ii] = taps[:, d][:, None]
        # j=0: w0 = -12; rows r=s+12, coefficients from left probes e_s
        # (support s <= i+12 <= 115); rows r < 12 hit the zero pad.
        bands[:, 0, 12:, o * DWT : o * DWT + DWT] = out[:, :116, :DWT]
        # j=39: out t = 4056+i (i<40); band[r, i] = M[t, 4044+r]
        #   = out(e~_{76+r})[472+i] for r < 52; rows >= 52 hit the zero pad.
        bands[:, 2, :52, o * DWT : o * DWT + 40] = \
            out[:, WIN + 76 : WIN + 128, _LP - 40 :]
    return bands


def compute_losses(lo_last):
    """ortho loss from the last level's lo filter, matching reference."""
    lo = lo_last.astype(np.float64)
    lo_pad = np.pad(lo, ((0, 0), (1, 1)))
    lo_smooth = np.abs(lo_pad[:, 1:] - lo_pad[:, :-1]).mean()
    lo_n = lo / (np.linalg.norm(lo, axis=1, keepdims=True) + 1e-8)
    shift = 0.0
    for s in (1, 2, 3):
        sh = np.roll(lo_n, s, axis=1)
        shift = shift + np.abs(lo_n[:, :, None] * sh[:, None, :]).mean()
    amp = np.abs((lo_n * lo_n).sum(axis=1) - 1.0).mean()
    return REG * (shift + amp) + 0.1 * lo_smooth


# ------------------------------------------------------- numpy full forward
def forward_numpy(x, stat_w, stat_b, wg1_w, wg1_b, wg2_w, wg2_b,
                  gates_w, gates_b, attn1_w, attn1_b, attn2_w, attn2_b,
                  dtype=np.float64):
    """Pure numpy replication of the reference (for validation)."""
    los, his = compute_filters(x, stat_w, stat_b, wg1_w, wg1_b, wg2_w, wg2_b)
    xd = x.astype(dtype)
    f = lambda a: a[:, None, :].astype(dtype)
    a1 = _corr_rep(xd, f(los[0])); d1 = _corr_rep(xd, f(his[0]))
    a2 = _corr_rep(a1, f(los[1])); d2 = _corr_rep(a1, f(his[1]))
    a3 = _corr_rep(a2, f(los[2])); d3 = _corr_rep(a2, f(his[2]))
    details = [d1, d2, d3]

    def conv1d(xx, w, b, pad):
        w = w.astype(dtype)
        cout, cin, K = w.shape
        xp = np.pad(xx, ((0, 0), (0, 0), (pad, pad)))
        n = xx.shape[2]
        out = np.zeros((xx.shape[0], cout, n), dtype=dtype)
        for k in range(K):
            out += np.einsum("oc,bct->bot", w[:, :, k], xp[:, :, k : k + n])
        return out + b.astype(dtype)[None, :, None]

    sig = lambda z: 1.0 / (1.0 + np.exp(-z))
    current = a3
    enhanced = []
    for i in range(LEVEL - 1, -1, -1):
        det = details[i]
        if i < LEVEL - 1:
            a = conv1d(current, attn1_w[i], attn1_b[i], 0)
            a = 0.5 * a * (1.0 + _erf(a / math.sqrt(2.0)))
            attn = sig(conv1d(a, attn2_w[i], attn2_b[i], 0))
            det = det * attn + det
        gate = sig(conv1d(current, gates_w[i], gates_b[i], 1))
        current = current + gate * det
        enhanced.append(det)
    enhanced.reverse()
    yl = current
    ortho = compute_losses(los[2])
    lo_all = np.stack(los).astype(np.float32).transpose(0, 1, 2)
    hi_all = np.stack(his).astype(np.float32)
    return (yl, enhanced[0], enhanced[1], enhanced[2],
            np.float32(ortho), np.float32(0.0),
            np.stack(los).astype(np.float32), np.stack(his).astype(np.float32))


# ================================================================ device ==
KW = 4100          # per-kind column stride in the staging buffer
DOFF = 2           # data starts at this column (4B alignment for fp16 ops)
SQ2 = 0.7071067811865476

_PROG_CACHE = {}

# engine assignment knobs (tuned via TimelineSim)
CFG = {
    "tt_pool": False,     # t-mults & cur2-add on GPSIMD
    "dp_pool": False,     # dp-STT on GPSIMD
    "yl_pool": False,     # STT is not a valid Pool opcode on HW (walrus)
    "copy_dve_of5": 2,    # dw copies: j%copy_mod < this -> DVE else ACT
    "copy_mod": 5,
    "ring_bufs": 4,
    "dwp": 2, "gp": 2, "a1p": 2, "a2p": 2,
}


def _build_program(k1, k2, k3, a1b_zero):
    """Build the SPMD single-core Bass program. k1/k2/k3 are log2 of the
    det1 / det2 / (det3,app3) scale factors baked into the band inputs."""
    from contextlib import ExitStack

    import concourse.bacc as bacc
    import concourse.bass as bass
    import concourse.tile as tile
    from concourse import mybir

    dt = mybir.dt
    AF = mybir.ActivationFunctionType
    f16, f32 = dt.float16, dt.float32
    s_d1, s_d2, s_3 = 2.0 ** k1, 2.0 ** k2, 2.0 ** k3

    nc = bacc.Bacc("TRN2", target_bir_lowering=False, debug=False)
    x_d = nc.dram_tensor("x16", [NS, C, L], f16, kind="ExternalInput").ap()
    bands_d = nc.dram_tensor("bands", [C, NS * 3 * 416], f16,
                             kind="ExternalInput").ap()
    gw_d = nc.dram_tensor("gw", [C, 9 * C], f16, kind="ExternalInput").ap()
    a1w_d = nc.dram_tensor("a1w", [C, 64], f16, kind="ExternalInput").ap()
    a2w_d = nc.dram_tensor("a2w", [C, 256], f16, kind="ExternalInput").ap()
    gb_d = nc.dram_tensor("gb", [C, 3], f32, kind="ExternalInput").ap()
    a1be_d = nc.dram_tensor("a1be", [C, 2], f32, kind="ExternalInput").ap()
    a1bs_d = nc.dram_tensor("a1bs", [C, 2], f32, kind="ExternalInput").ap()
    a2b_d = nc.dram_tensor("a2b", [C, 2], f32, kind="ExternalInput").ap()
    outs_d = {n: nc.dram_tensor(n, [NS, C, L], f16, kind="ExternalOutput").ap()
              for n in ("yl", "yh1", "yh2", "yh3")}

    with tile.TileContext(nc) as tc, ExitStack() as ctx:
        pool = lambda name, bufs, **kw: ctx.enter_context(
            tc.tile_pool(name=name, bufs=bufs, **kw))
        konst = pool("konst", 1)
        xp = pool("xp", 2)
        xcp = pool("xcp", 1)
        xtp = pool("xtp", 2)
        bigp = pool("bigp", 2)
        curp = pool("curp", 2)
        ring = pool("ring", CFG["ring_bufs"])
        dwp = pool("dwp", CFG["dwp"], space="PSUM")
        gp = pool("gp", 1, space="PSUM")  # one 2-bank pair tile
        a1p = pool("a1p", CFG["a1p"], space="PSUM")
        a2p = pool("a2p", CFG["a2p"], space="PSUM")

        def kload(name, shape, dtype, src):
            t = konst.tile(shape, dtype, tag=name)
            nc.sync.dma_start(t[:], src)
            return t

        bands_sb = kload("bands", [C, NS * 3 * 416], f16, bands_d)
        gw_sb = kload("gw", [C, 9 * C], f16, gw_d)
        a1w_sb = kload("a1w", [C, 64], f16, a1w_d)
        a2w_sb = kload("a2w", [C, 256], f16, a2w_d)
        gb_sb = kload("gb", [C, 3], f32, gb_d)
        a1be_sb = kload("a1be", [C, 2], f32, a1be_d)
        a1bs_sb = kload("a1bs", [C, 2], f32, a1bs_d)
        a2b_sb = kload("a2b", [C, 2], f32, a2b_d)

        ALU = mybir.AluOpType
        for s in range(NS):
            x_sb = xp.tile([C, XBUF], f16, tag="x")
            nc.gpsimd.memset(x_sb[:, 0:XLEAD], 0.0)
            nc.gpsimd.memset(x_sb[:, XLEAD + L :], 0.0)
            nc.sync.dma_start(x_sb[:, XLEAD : XLEAD + L], x_d[s])
            big = bigp.tile([C, 4 * KW], f16, tag="big")
            bigv = big[:].rearrange("p (k n) -> p k n", k=4)
            cur2 = curp.tile([C, KW], f16, tag="cur2")
            cur1 = curp.tile([C, KW], f16, tag="cur1")
            for t in (cur2[:], cur1[:]):
                nc.gpsimd.memset(t[:, DOFF - 1 : DOFF], 0.0)
                nc.gpsimd.memset(t[:, DOFF + L : DOFF + L + 1], 0.0)
            nc.gpsimd.memset(bigv[:, :, DOFF - 1 : DOFF], 0.0)
            nc.gpsimd.memset(bigv[:, :, DOFF + L : DOFF + L + 1], 0.0)

            # -------- depthwise pass: overlap-copy + blocked transpose +
            # one band matmul per 104-wide tile (no halo matmuls)
            xcat = xcp.tile([C, NDW * WIN], f16, tag="xcat")
            xcv = xcat[:].rearrange("p (j i) -> p j i", j=NDW)
            # overlapped windows: block j = xbuf[:, 104j : 104j+128]
            xap = x_sb[:]
            ov = bass.AP(xap.tensor, xap.offset,
                         [list(xap.ap[0]), [DWT, NDW], [1, WIN]])
            nc.sync.dma_start(xcv, ov)
            xt = xtp.tile([C, NDW * WIN], f16, tag="xt")
            xtv = xt[:].rearrange("p (j c) -> p j c", j=NDW)
            nc.sync.dma_start_transpose(xtv, xcat[:])
            for j in range(NDW):
                dq = dwp.tile([C, 4 * DWT], f32, tag="dw")
                tmpl = 0 if j == 0 else (2 if j == NDW - 1 else 1)
                nc.tensor.matmul(
                    dq[:], xtv[:, j, :],
                    bands_sb[:, (s * 3 + tmpl) * 416 : (s * 3 + tmpl + 1) * 416],
                    start=True, stop=True)
                wj = DWT if j < NDW - 1 else 40
                src_ap = dq[:, 0 : 4 * wj].rearrange("p (i k) -> p k i", k=4)
                dst = bigv[:, :, DOFF + DWT * j : DOFF + DWT * j + wj]
                if j % CFG["copy_mod"] < CFG["copy_dve_of5"]:
                    nc.vector.tensor_copy(dst, src_ap)
                else:
                    nc.scalar.copy(dst, src_ap)

            app3 = bigv[:, 3, :]
            dets = [bigv[:, 0, :], bigv[:, 1, :], bigv[:, 2, :]]

            gpair = {}

            def gate_conv(lvl, curbuf, t0, inv_scale):
                # pair adjacent csf tiles into one 2-bank psum tile so each
                # sigmoid covers 1024 cols (halves ACT per-op overhead)
                i = t0 // CSF
                pk = (lvl, i // 2)
                if pk not in gpair:
                    gq = gp.tile([C, 2 * CSF], f32, tag="g")
                    for h in range(2):
                        tb = CSF * (2 * (i // 2) + h)
                        for k in range(3):
                            nc.tensor.matmul(
                                gq[:, h * CSF : (h + 1) * CSF],
                                gw_sb[:, (lvl * 3 + k) * C : (lvl * 3 + k + 1) * C],
                                curbuf[:, tb + k + 1 : tb + k + 1 + CSF],
                                start=(k == 0), stop=(k == 2))
                    gt = ring.tile([C, 2 * CSF], f16, tag="gt")
                    nc.scalar.activation(gt[:], gq[:], AF.Sigmoid,
                                         bias=gb_sb[:, lvl : lvl + 1],
                                         scale=inv_scale)
                    gpair[pk] = gt
                return gpair[pk][:, (i % 2) * CSF : (i % 2 + 1) * CSF]

            # ---------------- level 2 (no attention)
            for i in range(NCT):
                t0 = CSF * i
                g2 = gate_conv(2, app3, t0, 1.0 / s_3)
                t2 = ring.tile([C, CSF], f16, tag="tt")
                vej = nc.gpsimd if CFG["tt_pool"] else nc.vector
                vej.tensor_mul(t2[:], g2[:],
                               dets[2][:, DOFF + t0 : DOFF + t0 + CSF])
                vej.tensor_add(cur2[:, DOFF + t0 : DOFF + t0 + CSF],
                               t2[:], app3[:, DOFF + t0 : DOFF + t0 + CSF])

            # ---------------- levels 1 and 0 (attention + gate)
            # dp overwrites the (dead) det slot in big; yl overwrites app3.
            for lvl, curi, curo, s_in, s_out in (
                    (1, cur2, cur1, s_3, s_d2), (0, cur1, None, s_d2, s_d1)):
                det = dets[lvl]
                asb = None
                for i in range(NCT):
                    t0 = CSF * i
                    if i % 4 == 0:
                        aq = a1p.tile([C, CSF], f32, tag="a1")
                        for g in range(4):
                            tg = CSF * (i + g)
                            nc.tensor.matmul(
                                aq[32 * g : 32 * (g + 1), :],
                                a1w_sb[:, lvl * 32 : (lvl + 1) * 32],
                                curi[:, DOFF + tg : DOFF + tg + CSF],
                                tile_position=(0, 32 * g),
                                start=True, stop=True, skip_group_check=(g > 0))
                        erf_t = ring.tile([C, CSF], f16, tag="erf")
                        nc.scalar.activation(erf_t[:], aq[:], AF.Erf,
                                             bias=a1be_sb[:, lvl : lvl + 1],
                                             scale=SQ2 / s_in)
                        asb = ring.tile([C, CSF], f16, tag="asb")
                        if a1b_zero:
                            nc.vector.scalar_tensor_tensor(
                                asb[:], erf_t[:], 1.0, aq[:],
                                op0=ALU.add, op1=ALU.mult)
                        else:
                            pre2 = ring.tile([C, CSF], f32, tag="pre2")
                            nc.vector.tensor_scalar(
                                pre2[:], aq[:], a1bs_sb[:, lvl : lvl + 1], None,
                                op0=ALU.add)
                            nc.vector.scalar_tensor_tensor(
                                asb[:], erf_t[:], 1.0, pre2[:],
                                op0=ALU.add, op1=ALU.mult)
                    gx = i % 4
                    a2q = a2p.tile([C, CSF], f32, tag="a2")
                    nc.tensor.matmul(a2q[:],
                                     a2w_sb[32 * gx : 32 * (gx + 1),
                                            lvl * C : (lvl + 1) * C],
                                     asb[32 * gx : 32 * (gx + 1), :],
                                     tile_position=(32 * gx, 0),
                                     start=True, stop=True)
                    attnw = ring.tile([C, CSF], f16, tag="attnw")
                    nc.scalar.activation(attnw[:], a2q[:], AF.Sigmoid,
                                         bias=a2b_sb[:, lvl : lvl + 1],
                                         scale=1.0 / s_in)
                    dp = det[:, DOFF + t0 : DOFF + t0 + CSF]
                    (nc.gpsimd if CFG["dp_pool"] else nc.vector).scalar_tensor_tensor(
                        dp, attnw[:], 1.0, dp,
                        op0=ALU.add, op1=ALU.mult)
                    g1 = gate_conv(lvl, curi, t0, 1.0 / s_in)
                    t1 = ring.tile([C, CSF], f16, tag="tt")
                    (nc.gpsimd if CFG["tt_pool"] else nc.vector).tensor_mul(
                        t1[:], g1[:], dp)
                    if lvl == 1:
                        nc.vector.scalar_tensor_tensor(
                            curo[:, DOFF + t0 : DOFF + t0 + CSF],
                            curi[:, DOFF + t0 : DOFF + t0 + CSF], s_out / s_in,
                            t1[:], op0=ALU.mult, op1=ALU.add)
                    else:
                        (nc.gpsimd if CFG["yl_pool"] else nc.vector).scalar_tensor_tensor(
                            app3[:, DOFF + t0 : DOFF + t0 + CSF],
                            curi[:, DOFF + t0 : DOFF + t0 + CSF],
                            s_out / s_in, t1[:], op0=ALU.mult, op1=ALU.add)
            nc.sync.dma_start(outs_d["yl"][s], app3[:, DOFF : DOFF + L])
            nc.sync.dma_start(outs_d["yh1"][s], dets[0][:, DOFF : DOFF + L])
            nc.sync.dma_start(outs_d["yh2"][s], dets[1][:, DOFF : DOFF + L])
            nc.sync.dma_start(outs_d["yh3"][s], dets[2][:, DOFF : DOFF + L])

    nc.compile()
    return nc


def _get_program(k1, k2, k3, a1b_zero):
    key = (k1, k2, k3, a1b_zero)
    if key not in _PROG_CACHE:
        _PROG_CACHE[key] = _build_program(k1, k2, k3, a1b_zero)
    return _PROG_CACHE[key]


def prepare(x, stat_w, stat_b, wg1_w, wg1_b, wg2_w, wg2_b,
            gates_w, gates_b, attn1_w, attn1_b, attn2_w, attn2_b):
    """Host-side prep: filters, losses, scaled bands, weight layouts,
    per-core input maps. Returns (progkey, in_maps, unscale, extras)."""
    x = np.asarray(x)
    los, his = compute_filters(x, stat_w, stat_b, wg1_w, wg1_b, wg2_w, wg2_b)
    ortho = compute_losses(los[2])
    lo_all = np.stack(los).astype(np.float32)
    hi_all = np.stack(his).astype(np.float32)

    bands = build_bands(los, his)                    # [B, 3, 128, 416] f64
    std_x = float(x.std())
    ks = []
    for o in range(4):
        # center column of the interior template holds the full 25-tap set
        tapn = np.sqrt((bands[:, 1, :, o * DWT + 52] ** 2).sum(axis=-1)).max()
        est = max(tapn * std_x * 5.0, 1e-300)
        ks.append(int(np.clip(round(math.log2(2.0 / est)), -100, 100)))
    k1 = ks[0]
    k2 = ks[1]
    k3 = min(ks[2], ks[3])
    scl = np.array([2.0 ** k1, 2.0 ** k2, 2.0 ** k3, 2.0 ** k3])
    bands = bands * np.repeat(scl, DWT)[None, None, None, :]
    # reorder band columns kind-major -> i-major (col = i*4 + kind)
    Bn = bands.shape[0]
    bands = bands.reshape(Bn, 3, WIN, 4, DWT).transpose(0, 1, 2, 4, 3)
    bands = np.ascontiguousarray(bands).reshape(Bn, 3, WIN, 4 * DWT)
    a1b_zero = bool(np.all(np.asarray(attn1_b) == 0.0))

    gw = np.ascontiguousarray(
        np.asarray(gates_w).transpose(2, 0, 3, 1).reshape(C, 9 * C)
    ).astype(np.float16)
    a1w = np.ascontiguousarray(
        np.asarray(attn1_w)[:, :, :, 0].transpose(2, 0, 1).reshape(C, 64)
    ).astype(np.float16)
    a2w_eff = 0.5 * np.asarray(attn2_w)[:, :, :, 0]       # [2, 128, 32]
    a2w = np.ascontiguousarray(
        np.tile(a2w_eff.transpose(2, 0, 1).reshape(32, 2 * C), (4, 1))
    ).astype(np.float16)
    gb = np.ascontiguousarray(np.asarray(gates_b).T).astype(np.float32)
    a2b = np.ascontiguousarray(np.asarray(attn2_b).T).astype(np.float32)
    a1t = np.tile(np.asarray(attn1_b), (1, 4))            # [2, 128]
    a1be = np.ascontiguousarray((a1t * SQ2).T).astype(np.float32)
    a1bs = np.ascontiguousarray(
        (a1t * np.array([[2.0 ** k2], [2.0 ** k3]])).T).astype(np.float32)

    in_maps = []
    for c in range(NCORES):
        sl = slice(c * NS, (c + 1) * NS)
        bsh = bands[sl].transpose(2, 0, 1, 3).reshape(C, NS * 3 * 416)
        in_maps.append({
            "x16": np.ascontiguousarray(x[sl]).astype(np.float16),
            "bands": np.ascontiguousarray(bsh).astype(np.float16),
            "gw": gw, "a1w": a1w, "a2w": a2w, "gb": gb,
            "a1be": a1be, "a1bs": a1bs, "a2b": a2b,
        })
    unscale = {"yl": 2.0 ** -k1, "yh1": 2.0 ** -k1,
               "yh2": 2.0 ** -k2, "yh3": 2.0 ** -k3}
    return (k1, k2, k3, a1b_zero), in_maps, unscale, (ortho, lo_all, hi_all)


def kernel(x, stat_w, stat_b, wg1_w, wg1_b, wg2_w, wg2_b,
           gates_w, gates_b, attn1_w, attn1_b, attn2_w, attn2_b):
    from concourse.bass_utils import run_bass_kernel_spmd

    args = [np.asarray(a) for a in (
        x, stat_w, stat_b, wg1_w, wg1_b, wg2_w, wg2_b,
        gates_w, gates_b, attn1_w, attn1_b, attn2_w, attn2_b)]
    key, in_maps, unscale, (ortho, lo_all, hi_all) = prepare(*args)
    nc = _get_program(*key)
    res = run_bass_kernel_spmd(nc, in_maps, list(range(NCORES)))
    full = {}
    for name in ("yl", "yh1", "yh2", "yh3"):
        parts = [res.results[c][name].astype(np.float32) * unscale[name]
                 for c in range(NCORES)]
        full[name] = np.concatenate(parts, axis=0)
    return (full["yl"], full["yh1"], full["yh2"], full["yh3"],
            np.float32(ortho), np.float32(0.0), lo_all, hi_all)


if __name__ == "__main__":
    print("kernel.py module loaded OK")
